# revision 1
# baseline (speedup 1.0000x reference)
"""CamCenterLoss (segment-mean SmoothL1) on 8 Trainium2 NeuronCores.

Sharding: by camera id (8 cams = 8 cores), so every (label, cam) segment is
fully local to one core and no collective is needed.

Key trick vs the naive 3-matmul scheme: targets - feats = (P - I) @ feats
where P is the block-local averaging projector (P[i,j] = 1/c if rows i,j in
the same segment else 0).  M = P - I is precomputed on the host per 128-row
block, so the device does ONE [128x128] @ [128x2048] matmul per block.
Singleton segments (count==1) have d == 0 and are dropped on the host.

SmoothL1 identity used on device (a = |d|, m = min(a,1)):
  sl1(d) = a - m + 0.5*m^2
  partial = Sum(a) - Sum(m) + 0.5*Sum(m^2)
Per block:
  PE   : d = M^T @ fe            (4 matmuls of N=512 into one 4-bank PSUM tile)
  ACT  : a = Abs(d) + accum -> Sum_a   (only ACT can do abs; drains PSUM)
  DVE  : m = min(a,1) + accum -> Sum_m (tensor_scalar, 4x mode)
  DVE+GPS: q = m*m                     (tensor_tensor, 2x; GPSIMD does a
           slice -- it cannot read PSUM or run tensor_scalar, but tt works)
  DVE  : sum pass over q + accum -> Sum_q (tensor_scalar mult 1.0, 4x mode)
All sums are free per-partition accumulator outputs; scalar_tensor_tensor,
pow and abs_max are rejected by the walrus ISA verifier, hence this split.
"""

import numpy as np
import ml_dtypes

N_CORES = 8
NUM_CAMS = 8
D_FEAT = 2048
QCHUNK = 512
GCOL = 384           # columns of the q=m*m pass done by GPSIMD


# ----------------------------------------------------------------------------
# Host-side preprocessing (index manipulation + row permutation + dtype cast)
# ----------------------------------------------------------------------------

def _preprocess(feats, labels, cam_ids):
    feats = np.ascontiguousarray(np.asarray(feats, dtype=np.float32))
    labels = np.asarray(labels).astype(np.int64)
    cams = np.asarray(cam_ids).astype(np.int64)
    N, D = feats.shape

    per_core = []
    for c in range(N_CORES):
        rows = np.flatnonzero(cams == c)
        segs = labels[rows]
        order = np.argsort(segs, kind="stable")
        rows = rows[order]
        segs = segs[order]
        n = len(rows)
        starts = np.flatnonzero(np.r_[True, segs[1:] != segs[:-1]])
        ends = np.r_[starts[1:], n]
        # drop singleton segments: d == 0, contributes 0 to the loss
        runs = [(s, e) for s, e in zip(starts, ends) if e - s >= 2]
        blocks = []
        cur, used = [], 0
        for s, e in runs:
            rl = e - s
            if rl > 128:
                raise ValueError("segment with more than 128 rows")
            if used + rl > 128:
                blocks.append(cur)
                cur, used = [], 0
            cur.append((s, e))
            used += rl
        if cur:
            blocks.append(cur)
        per_core.append((rows, blocks))

    nblk = max(max((len(b) for _, b in per_core), default=1), 1)

    bf16 = ml_dtypes.bfloat16
    feats_s = np.zeros((N_CORES, nblk * 128, D), dtype=bf16)
    m_mat32 = np.zeros((N_CORES, nblk, 128, 128), dtype=np.float32)

    for c in range(N_CORES):
        rows, blocks = per_core[c]
        for b, blist in enumerate(blocks):
            k = 0
            for (s, e) in blist:
                cnt = e - s
                ridx = rows[s:e]
                feats_s[c, 128 * b + k:128 * b + k + cnt] = feats[ridx]
                m_mat32[c, b, k:k + cnt, k:k + cnt] = 1.0 / cnt
                for j in range(k, k + cnt):
                    m_mat32[c, b, j, j] -= 1.0
                k += cnt
    m_mat = m_mat32.astype(bf16)
    return feats_s, m_mat, nblk, N, D


# ----------------------------------------------------------------------------
# Device program
# ----------------------------------------------------------------------------

def _build_program(nblk, D):
    import concourse.bacc as bacc
    import concourse.mybir as mybir
    import concourse.tile as tile

    dt = mybir.dt
    f32, bf16 = dt.float32, dt.bfloat16
    Alu = mybir.AluOpType
    Act = mybir.ActivationFunctionType

    nc = bacc.Bacc("TRN2", target_bir_lowering=False, debug=False,
                   num_devices=N_CORES)
    feats_d = nc.dram_tensor("feats_s", [nblk * 128, D], bf16,
                             kind="ExternalInput").ap()
    mmat_d = nc.dram_tensor("m_mat", [nblk, 128, 128], bf16,
                            kind="ExternalInput").ap()
    out_d = nc.dram_tensor("partial", [1, 1], f32, kind="ExternalOutput").ap()

    with tile.TileContext(nc) as tc:
        with (
            tc.tile_pool(name="const", bufs=1) as const_pool,
            tc.tile_pool(name="feats", bufs=4) as feats_pool,
            tc.tile_pool(name="wts", bufs=4) as wts_pool,
            tc.tile_pool(name="aa", bufs=3) as a_pool,
            tc.tile_pool(name="mm", bufs=2) as m_pool,
            tc.tile_pool(name="qq", bufs=2) as q_pool,
            tc.tile_pool(name="psumd", bufs=2, space="PSUM") as psum_d_pool,
        ):
            stats_aa = const_pool.tile([128, nblk], f32, tag="stats_aa")
            stats_m = const_pool.tile([128, nblk], f32, tag="stats_m")
            stats_q = const_pool.tile([128, nblk], f32, tag="stats_q")
            ones = const_pool.tile([128, 1], f32, tag="ones")
            nc.gpsimd.memset(ones[:], 1.0)

            for b in range(nblk):
                fe = feats_pool.tile([128, D], bf16, tag="fe")
                nc.sync.dma_start(fe[:], feats_d[128 * b:128 * (b + 1), :])
                mt = wts_pool.tile([128, 128], bf16, tag="mt")
                nc.sync.dma_start(mt[:], mmat_d[b])

                dps = psum_d_pool.tile([128, D], f32, tag="d")
                for q in range(D // QCHUNK):
                    sl = slice(q * QCHUNK, (q + 1) * QCHUNK)
                    nc.tensor.matmul(dps[:, sl], mt[:], fe[:, sl],
                                     start=True, stop=True)

                # NOTE tensor_scalar+accum_out semantics: out = op0(in, s1),
                # accum_out = add-reduce(out) via op1=add.
                a = a_pool.tile([128, D], bf16, tag="a")
                nc.scalar.activation(a[:], dps[:], Act.Abs,
                                     accum_out=stats_aa[:, b:b + 1])

                m = m_pool.tile([128, D], bf16, tag="m")
                nc.vector.tensor_scalar(m[:], a[:], 1.0, None, op0=Alu.min,
                                        op1=Alu.add,
                                        accum_out=stats_m[:, b:b + 1])
                qq = q_pool.tile([128, D], bf16, tag="q")
                nc.gpsimd.tensor_tensor(qq[:, 0:GCOL], m[:, 0:GCOL],
                                        m[:, 0:GCOL], op=Alu.mult)
                nc.vector.tensor_tensor(qq[:, GCOL:D], m[:, GCOL:D],
                                        m[:, GCOL:D], op=Alu.mult)
                qs = q_pool.tile([128, D], bf16, tag="qs")
                nc.vector.tensor_scalar(qs[:], qq[:], 1.0, None, op0=Alu.mult,
                                        op1=Alu.add,
                                        accum_out=stats_q[:, b:b + 1])

            # partial = Sum_a - Sum_m + 0.5*Sum_q, then across partitions
            t2 = const_pool.tile([128, nblk], f32, tag="t2")
            nc.vector.tensor_tensor(t2[:], stats_aa[:], stats_m[:],
                                    op=Alu.subtract)
            comb = const_pool.tile([128, nblk], f32, tag="comb")
            nc.vector.scalar_tensor_tensor(comb[:], stats_q[:], 0.5, t2[:],
                                           op0=Alu.mult, op1=Alu.add)
            red = const_pool.tile([128, 1], f32, tag="red")
            nc.vector.tensor_reduce(red[:], comb[:],
                                    axis=mybir.AxisListType.X, op=Alu.add)
            fin = psum_d_pool.tile([1, 1], f32, tag="d")
            nc.tensor.matmul(fin[:], red[:], ones[:], start=True, stop=True)
            outsb = const_pool.tile([1, 1], f32, tag="outsb")
            nc.scalar.copy(outsb[:], fin[:])
            nc.sync.dma_start(out_d[:], outsb[:])

    nc.compile()
    return nc


_PROGRAM_CACHE = {}


def _get_program(nblk, D):
    key = (nblk, D)
    if key not in _PROGRAM_CACHE:
        _PROGRAM_CACHE[key] = _build_program(nblk, D)
    return _PROGRAM_CACHE[key]


def make_in_maps(feats, labels, cam_ids):
    """Host shard + program build; returns (nc, in_maps, N, D)."""
    feats_s, m_mat, nblk, N, D = _preprocess(feats, labels, cam_ids)
    nc = _get_program(nblk, D)
    in_maps = [
        {"feats_s": feats_s[c], "m_mat": m_mat[c]}
        for c in range(N_CORES)
    ]
    return nc, in_maps, N, D


def kernel(feats, labels, cam_ids):
    from concourse.bass_utils import run_bass_kernel_spmd

    nc, in_maps, N, D = make_in_maps(feats, labels, cam_ids)
    res = run_bass_kernel_spmd(nc, in_maps, core_ids=list(range(N_CORES)))
    total = np.sum(
        np.array([res.results[c]["partial"][0, 0] for c in range(N_CORES)],
                 dtype=np.float64))
    return np.float32(total / (float(N) * float(D)))



# revision 5
# speedup vs baseline: 1.6938x; 1.6938x over previous
"""CamCenterLoss (segment-mean SmoothL1) on 8 Trainium2 NeuronCores.

Sharding: each (label, cam) segment is assigned wholly to one core.
Segments (size>=2; singletons contribute 0) are packed into 128-row
blocks with best-fit-decreasing, and blocks are dealt across the 8
cores so every core gets the same block count (nblk ~ 14).

Per block the device computes d = M^T @ fe where M = P - I is the
block-local averaging projector built on the host (P[i,j] = 1/c if
rows i,j in the same segment else 0), so targets - feats needs ONE
[128x128] @ [128x2048] matmul per block.

SmoothL1 identity used on device (a = |d|, m = min(a, 1)):
    sl1 = a - (m - 0.5*m^2) = a + 0.5*((m - 2) * m)
so the loss partial needs only two sums, each riding a fused op:
  PE  : d = M^T @ fe                      (4 matmuls of N=512, 1 PSUM tile)
  ACT : a = Abs(d), accum Sum_a           (drains PSUM -> bf16 SBUF)
  DVE : m = min(a, 1)                     (tensor_scalar, 4x mode)
  DVE : v = (m sub 2) mult m, accum Sum_v (scalar_tensor_tensor)
  partial = Sum_a + 0.5 * Sum_v
(tensor_tensor_reduce dies at runtime on HW; tensor_scalar with accum_out
drops to 1x mode — hence exactly one accum-free 4x op plus one stt.)
"""

import numpy as np
import ml_dtypes

N_CORES = 8
NUM_CAMS = 8
NUM_LABELS = 1024
D_FEAT = 2048
QCHUNK = 512


# ----------------------------------------------------------------------------
# Host-side preprocessing (index manipulation + row permutation + dtype cast)
# ----------------------------------------------------------------------------

def _preprocess(feats, labels, cam_ids):
    feats = np.ascontiguousarray(np.asarray(feats, dtype=np.float32))
    labels = np.asarray(labels).astype(np.int64)
    cams = np.asarray(cam_ids).astype(np.int64)
    N, D = feats.shape

    # Global segment id; gather row lists per segment with one argsort.
    seg = labels * NUM_CAMS + cams
    order = np.argsort(seg, kind="stable")
    seg_sorted = seg[order]
    starts = np.flatnonzero(np.r_[True, seg_sorted[1:] != seg_sorted[:-1]])
    ends = np.r_[starts[1:], N]
    # Keep segments with >= 2 rows; singletons have d == 0.
    runs = [(e - s, s) for s, e in zip(starts, ends) if e - s >= 2]
    if any(rl > 128 for rl, _ in runs):
        raise ValueError("segment with more than 128 rows")

    # Best-fit-decreasing pack into 128-row bins.
    runs.sort(reverse=True)
    bins = []          # list of (used, [(start, len), ...])
    for rl, s in runs:
        best_i, best_used = -1, -1
        for i, (used, _) in enumerate(bins):
            if used + rl <= 128 and used > best_used:
                best_i, best_used = i, used
        if best_i < 0:
            bins.append((rl, [(s, rl)]))
        else:
            used, lst = bins[best_i]
            lst.append((s, rl))
            bins[best_i] = (used + rl, lst)

    nbins = len(bins)
    nblk = -(-nbins // N_CORES)
    nblk = max(nblk, 1)

    bf16 = ml_dtypes.bfloat16
    feats_s = np.zeros((N_CORES, nblk * 128, D), dtype=bf16)
    m_mat32 = np.zeros((N_CORES, nblk, 128, 128), dtype=np.float32)

    for i, (_, lst) in enumerate(bins):
        c, b = i % N_CORES, i // N_CORES
        k = 0
        for (s, rl) in lst:
            ridx = order[s:s + rl]
            feats_s[c, 128 * b + k:128 * b + k + rl] = feats[ridx]
            blk = m_mat32[c, b]
            blk[k:k + rl, k:k + rl] = 1.0 / rl
            for j in range(k, k + rl):
                blk[j, j] -= 1.0
            k += rl
    m_mat = m_mat32.astype(bf16)
    return feats_s, m_mat, nblk, N, D


# ----------------------------------------------------------------------------
# Device program
# ----------------------------------------------------------------------------

def _build_program(nblk, D):
    import concourse.bacc as bacc
    import concourse.mybir as mybir
    import concourse.tile as tile

    dt = mybir.dt
    f32, bf16 = dt.float32, dt.bfloat16
    Alu = mybir.AluOpType
    Act = mybir.ActivationFunctionType

    nc = bacc.Bacc("TRN2", target_bir_lowering=False, debug=False,
                   num_devices=N_CORES)
    feats_d = nc.dram_tensor("feats_s", [nblk * 128, D], bf16,
                             kind="ExternalInput").ap()
    mmat_d = nc.dram_tensor("m_mat", [nblk, 128, 128], bf16,
                            kind="ExternalInput").ap()
    out_d = nc.dram_tensor("partial", [1, 1], f32, kind="ExternalOutput").ap()

    with tile.TileContext(nc) as tc:
        with (
            tc.tile_pool(name="const", bufs=1) as const_pool,
            tc.tile_pool(name="feats", bufs=3) as feats_pool,
            tc.tile_pool(name="wts", bufs=3) as wts_pool,
            tc.tile_pool(name="aa", bufs=2) as a_pool,
            tc.tile_pool(name="tt", bufs=2) as t_pool,
            tc.tile_pool(name="uu", bufs=2) as u_pool,
            tc.tile_pool(name="psumd", bufs=2, space="PSUM") as psum_d_pool,
        ):
            stats_a = const_pool.tile([128, nblk], f32, tag="stats_a")
            stats_v = const_pool.tile([128, nblk], f32, tag="stats_v")
            ones1 = const_pool.tile([128, 1], f32, tag="ones1")
            nc.gpsimd.memset(ones1[:], 1.0)

            for b in range(nblk):
                fe = feats_pool.tile([128, D], bf16, tag="fe")
                nc.sync.dma_start(fe[:], feats_d[128 * b:128 * (b + 1), :])
                mt = wts_pool.tile([128, 128], bf16, tag="mt")
                nc.sync.dma_start(mt[:], mmat_d[b])

                dps = psum_d_pool.tile([128, D], f32, tag="d")
                for q in range(D // QCHUNK):
                    sl = slice(q * QCHUNK, (q + 1) * QCHUNK)
                    nc.tensor.matmul(dps[:, sl], mt[:], fe[:, sl],
                                     start=True, stop=True)

                a = a_pool.tile([128, D], bf16, tag="a")
                nc.scalar.activation(a[:], dps[:], Act.Abs,
                                     accum_out=stats_a[:, b:b + 1])

                # m = min(a, 1)  (4x mode: bf16, SBUF, no accum)
                m = t_pool.tile([128, D], bf16, tag="m")
                nc.vector.tensor_scalar_min(m[:], a[:], 1.0)

                # v = (m - 2) * m; accum -> Sum_v   (v itself is dead)
                v = u_pool.tile([128, D], bf16, tag="v")
                nc.vector.scalar_tensor_tensor(
                    v[:], m[:], 2.0, m[:], op0=Alu.subtract, op1=Alu.mult,
                    accum_out=stats_v[:, b:b + 1])

            # partial = Sum_a + 0.5 * Sum_v, then across partitions
            comb = const_pool.tile([128, nblk], f32, tag="comb")
            nc.vector.scalar_tensor_tensor(comb[:], stats_v[:], 0.5,
                                           stats_a[:], op0=Alu.mult,
                                           op1=Alu.add)
            red = const_pool.tile([128, 1], f32, tag="red")
            nc.vector.tensor_reduce(red[:], comb[:],
                                    axis=mybir.AxisListType.X, op=Alu.add)
            fin = psum_d_pool.tile([1, 1], f32, tag="d")
            nc.tensor.matmul(fin[:], red[:], ones1[:], start=True, stop=True)
            outsb = const_pool.tile([1, 1], f32, tag="outsb")
            nc.scalar.copy(outsb[:], fin[:])
            nc.sync.dma_start(out_d[:], outsb[:])

    nc.compile()
    return nc


_PROGRAM_CACHE = {}


def _get_program(nblk, D):
    key = (nblk, D)
    if key not in _PROGRAM_CACHE:
        _PROGRAM_CACHE[key] = _build_program(nblk, D)
    return _PROGRAM_CACHE[key]


def make_in_maps(feats, labels, cam_ids):
    """Host shard + program build; returns (nc, in_maps, N, D)."""
    feats_s, m_mat, nblk, N, D = _preprocess(feats, labels, cam_ids)
    nc = _get_program(nblk, D)
    in_maps = [
        {"feats_s": feats_s[c], "m_mat": m_mat[c]}
        for c in range(N_CORES)
    ]
    return nc, in_maps, N, D


def kernel(feats, labels, cam_ids):
    from concourse.bass_utils import run_bass_kernel_spmd

    nc, in_maps, N, D = make_in_maps(feats, labels, cam_ids)
    res = run_bass_kernel_spmd(nc, in_maps, core_ids=list(range(N_CORES)))
    total = np.sum(
        np.array([res.results[c]["partial"][0, 0] for c in range(N_CORES)],
                 dtype=np.float64))
    return np.float32(total / (float(N) * float(D)))
